# revision 10
# baseline (speedup 1.0000x reference)
"""AWQ W4A16 linear kernel for Trainium2 (8 NeuronCores, tensor-parallel).

y = x @ dequant(qweight, wscales, wzeros)^T + bias
  x:       [4096, 4096] fp32
  qweight: [12288, 2048] int32 (2 uint4 per value, low nibble = even k)
  wscales: [32, 12288] fp32   (per group of 128 k)
  wzeros:  [32, 12288] fp32
  bias:    [12288] fp32
  out:     [4096, 12288] fp32

Strategy: column-parallel across 8 cores (each core owns 1536 output
columns). Weights are dequantized host-side to fp16 in [K, N] layout; x is
transposed host-side to [K, M] fp16 (fp16 keeps the relative error at
~3e-4; bf16 would give ~2e-3). Each core runs a dense GEMM: output tiles
y[m:128, :1536] are accumulated over 32 k-tiles into 3 PSUM banks
(512 cols each), with the bias added by the DVE during PSUM->SBUF
eviction. The full weight slice (12.6 MB fp16) stays resident in SBUF;
x^T streams through in column blocks.
"""

import numpy as np

import concourse.mybir as mybir
import concourse.tile as tile
from concourse import bacc

M, K, N = 4096, 4096, 12288
GROUP = 128
NCORES = 8
NS = N // NCORES          # 1536 out columns per core
P = 128
KT = K // P               # 32 k tiles
NCHUNK = 512
NCH = NS // NCHUNK        # 3 psum banks per m tile
MBLK = 512                # m block held in SBUF at once
MB = M // MBLK            # 8
MSUB = MBLK // P          # 4

_DT = mybir.dt.float16
_NP_DT = np.float16


def _ldw_sig(ins):
    ap = ins.ins[0]
    return (
        ap.concise() if hasattr(ap, "concise") else str(ap),
        ins.perf_mode,
        ins.is_transpose,
        ins.tile_position,
        ins.tile_size,
    )


def _dedupe_ldweights(nc):
    """Drop InstLdweights that reload the exact weights already in the PE
    array (the walrus --enable-ldw-opt pass is broken in this toolchain).
    Conservative: only removes sync-free loads with an identical signature
    to the previous load, with nothing but matmuls in between."""
    mapping = {}
    for blk in nc.main_func.blocks:
        new_insts = []
        last_ldw = None
        for ins in blk.instructions:
            if isinstance(ins, mybir.InstLdweights):
                si = ins.sync_info
                clean = not si or (not si.on_wait and not si.on_update)
                sig = _ldw_sig(ins)
                if last_ldw is not None and clean and sig == last_ldw[0]:
                    mapping[ins.name] = last_ldw[1]
                    continue
                last_ldw = (sig, ins.name)
            elif isinstance(ins, mybir.InstMatmult):
                pass  # does not disturb loaded weights
            elif getattr(ins, "engine", None) == mybir.EngineType.PE:
                last_ldw = None
            new_insts.append(ins)
        blk.instructions[:] = new_insts
    if mapping:
        for blk in nc.main_func.blocks:
            for ins in blk.instructions:
                ins.remap_dependency_names(mapping)
        if hasattr(nc, "inst_map"):
            for name in mapping:
                nc.inst_map.pop(name, None)
    return len(mapping)


def _build(repeat=1, xbufs=2, xsplit=4, mblk=MBLK, dedupe=True):
    nc = _build_module(repeat, xbufs, xsplit, mblk)
    if dedupe:
        try:
            _dedupe_ldweights(nc)
        except Exception:
            nc = _build_module(repeat, xbufs, xsplit, mblk)
    nc.compile()
    return nc


def _build_module(repeat=1, xbufs=2, xsplit=4, mblk=MBLK):
    from contextlib import nullcontext

    mb_count = M // mblk
    msub = mblk // P

    nc = bacc.Bacc(None, target_bir_lowering=False)
    xt = nc.dram_tensor("xt", [K, M], _DT, kind="ExternalInput")
    wt = nc.dram_tensor("wt", [K, NS], _DT, kind="ExternalInput")
    bb = nc.dram_tensor("bb", [P, NS], mybir.dt.float32, kind="ExternalInput")
    y = nc.dram_tensor("y", [M, NS], mybir.dt.float32, kind="ExternalOutput")

    with tile.TileContext(nc) as tc:
        with (
            tc.tile_pool(name="wpool", bufs=1) as wpool,
            tc.tile_pool(name="bpool", bufs=1) as bpool,
            tc.tile_pool(name="xpool", bufs=xbufs) as xpool,
            tc.tile_pool(name="opool", bufs=2) as opool,
            tc.tile_pool(name="psum", bufs=2, space="PSUM") as psum,
        ):
            w_sb = wpool.tile([P, KT, NS], _DT)
            wt_r = wt.rearrange("(kt p) n -> p kt n", p=P)
            for kt in range(KT):
                nc.sync.dma_start(w_sb[:, kt, :], wt_r[:, kt, :])
            bias_sb = bpool.tile([P, NS], mybir.dt.float32)
            nc.sync.dma_start(bias_sb[:], bb[:, :])

            xt_r = xt.rearrange("(kt p) m -> p kt m", p=P)
            loop = tc.For_i(0, repeat, 1) if repeat != 1 else nullcontext()
            with loop:
                for mb in range(mb_count):
                    x_sb = xpool.tile([P, KT, mblk], _DT)
                    step = mblk // xsplit
                    for sp in range(xsplit):
                        o = sp * step
                        nc.sync.dma_start(
                            x_sb[:, :, o:o + step],
                            xt_r[:, :, mb * mblk + o:mb * mblk + o + step],
                        )
                    for ms in range(msub):
                        psts = [
                            psum.tile([P, NCHUNK], mybir.dt.float32,
                                      name=f"ps{i}")
                            for i in range(NCH)
                        ]
                        lhs = x_sb[:, :, ms * P:(ms + 1) * P]
                        for kt in range(KT):
                            for i in range(NCH):
                                nc.tensor.matmul(
                                    psts[i][:],
                                    lhs[:, kt, :],
                                    w_sb[:, kt, i * NCHUNK:(i + 1) * NCHUNK],
                                    start=(kt == 0),
                                    stop=(kt == KT - 1),
                                )
                        out_sb = opool.tile([P, NS], mybir.dt.float32)
                        for i in range(NCH):
                            sl = slice(i * NCHUNK, (i + 1) * NCHUNK)
                            nc.vector.tensor_add(
                                out_sb[:, sl], psts[i][:], bias_sb[:, sl]
                            )
                        m0 = mb * mblk + ms * P
                        nc.sync.dma_start(y[m0:m0 + P, :], out_sb[:, :])
    return nc


def _dequant_wt(qweight, wscales, wzeros):
    """Return w^T [K, N] fp16: w[n,k] = (wint[n,k] - z[g,n]) * s[g,n]."""
    qw = np.asarray(qweight).astype(np.int32)
    low = (qw & 0xF).astype(np.float32)          # [N, K//2] -> even k
    high = ((qw >> 4) & 0xF).astype(np.float32)  # odd k
    G = K // GROUP
    wiT = np.empty((K, qw.shape[0]), dtype=np.float32)
    wiT[0::2, :] = low.T
    wiT[1::2, :] = high.T
    wg = wiT.reshape(G, GROUP, -1)
    wg -= np.asarray(wzeros, dtype=np.float32)[:, None, :]
    wg *= np.asarray(wscales, dtype=np.float32)[:, None, :]
    return wg.reshape(K, -1).astype(_NP_DT)


def prepare_inputs(x, qweight, wscales, wzeros, bias):
    xt16 = np.asarray(x).T.astype(_NP_DT)  # [K, M]
    wt16 = _dequant_wt(qweight, wscales, wzeros)  # [K, N]
    bias = np.asarray(bias, dtype=np.float32)
    in_maps = []
    for c in range(NCORES):
        sl = slice(c * NS, (c + 1) * NS)
        in_maps.append({
            "xt": xt16,
            "wt": np.ascontiguousarray(wt16[:, sl]),
            "bb": np.ascontiguousarray(np.broadcast_to(bias[sl], (P, NS))),
        })
    return in_maps


class _Runner:
    """Compiled SPMD executable with cached jit; run(in_maps) -> y pieces."""

    def __init__(self, nc):
        import jax
        from jax.sharding import Mesh, PartitionSpec, NamedSharding
        from jax.experimental.shard_map import shard_map
        from concourse.bass2jax import (
            _bass_exec_p, install_neuronx_cc_hook, partition_id_tensor,
        )

        install_neuronx_cc_hook()
        self.jax = jax
        partition_name = (
            nc.partition_id_tensor.name if nc.partition_id_tensor else None
        )
        in_names, out_names, out_avals = [], [], []
        for alloc in nc.m.functions[0].allocations:
            if not isinstance(alloc, mybir.MemoryLocationSet):
                continue
            name = alloc.memorylocations[0].name
            if alloc.kind == "ExternalInput":
                if name != partition_name:
                    in_names.append(name)
            elif alloc.kind == "ExternalOutput":
                out_names.append(name)
                out_avals.append(
                    jax.core.ShapedArray(
                        tuple(alloc.tensor_shape), mybir.dt.np(alloc.dtype)
                    )
                )
        self.in_names, self.out_names, self.out_avals = (
            in_names, out_names, out_avals
        )
        all_names = in_names + out_names
        if partition_name is not None:
            all_names = all_names + [partition_name]

        def _body(*args):
            operands = list(args)
            if partition_name is not None:
                operands.append(partition_id_tensor())
            outs = _bass_exec_p.bind(
                *operands,
                out_avals=tuple(out_avals),
                in_names=tuple(all_names),
                out_names=tuple(out_names),
                lowering_input_output_aliases=(),
                sim_require_finite=True,
                sim_require_nnan=True,
                nc=nc,
            )
            return tuple(outs)

        devices = jax.devices()[:NCORES]
        mesh = Mesh(np.asarray(devices), ("core",))
        n_params = len(in_names)
        n_outs = len(out_names)
        # "xt" is identical on every core: mark it replicated so only one
        # copy crosses the host->device link.
        self.replicated = {"xt"}
        in_specs = tuple(
            PartitionSpec() if nm in self.replicated else PartitionSpec("core")
            for nm in in_names
        ) + (PartitionSpec("core"),) * n_outs
        self.sharded = jax.jit(
            shard_map(
                _body, mesh=mesh,
                in_specs=in_specs,
                out_specs=(PartitionSpec("core"),) * n_outs,
                check_rep=False,
            ),
            donate_argnums=tuple(range(n_params, n_params + n_outs)),
            keep_unused=True,
        )
        self.sharding = NamedSharding(mesh, PartitionSpec("core"))
        self.rep_sharding = NamedSharding(mesh, PartitionSpec())

    def run(self, in_maps):
        jax = self.jax
        concat_in = []
        for nm in self.in_names:
            if nm in self.replicated:
                concat_in.append(
                    jax.device_put(np.asarray(in_maps[0][nm]),
                                   self.rep_sharding)
                )
            else:
                concat_in.append(
                    jax.device_put(
                        np.concatenate(
                            [np.asarray(in_maps[c][nm])
                             for c in range(NCORES)], axis=0
                        ),
                        self.sharding,
                    )
                )
        zs = [
            jax.device_put(
                np.zeros((NCORES * av.shape[0], *av.shape[1:]), av.dtype),
                self.sharding,
            )
            for av in self.out_avals
        ]
        out = self.sharded(*concat_in, *zs)
        return {
            nm: np.asarray(out[i]).reshape(NCORES, *self.out_avals[i].shape)
            for i, nm in enumerate(self.out_names)
        }


_runner_cache = None


def _get_runner():
    global _runner_cache
    if _runner_cache is None:
        _runner_cache = _Runner(_build())
    return _runner_cache


def kernel(x, qweight, wscales, wzeros, bias):
    global _runner_cache
    in_maps = prepare_inputs(x, qweight, wscales, wzeros, bias)
    try:
        res = _get_runner().run(in_maps)
    except Exception:
        # One retry with a fresh runner (transient device/tunnel hiccups).
        _runner_cache = None
        res = _get_runner().run(in_maps)
    y = res["y"]  # [NCORES, M, NS]
    return np.ascontiguousarray(
        np.moveaxis(y, 0, 1).reshape(M, N)
    ).astype(np.float32)
